# revision 20
# baseline (speedup 1.0000x reference)
"""Trainium2 Bass kernel for nn_CrossImageAttentionLayer (v3).

Contract: kernel(**inputs) takes FULL unsharded inputs (as produced by
setup_inputs) and returns the FULL (B, N, D) output. Internally shards the
flattened (B*N) query dimension across 8 NeuronCores (all-reduce-free),
builds one SPMD Bass/Tile kernel, and runs it via run_bass_kernel_spmd.

v3 structure:
  - image features are repacked on the host into overlapping 2x2 pixel
    blocks in bf16: blk[c*4096 + y*64 + x] = feat[c, y:y+2, x:x+2] flat
    (a=yi, b=xi, d).  One bilinear footprint == one contiguous 2KB read,
    so each (query, camera) pair costs ONE descriptor.
  - valid-mask compaction: queries are sorted per core by their number of
    valid cameras and each query's valid cameras are compacted into the
    leading "slots".  Each 128-query tile then only gathers / computes
    max-valid-in-tile slots (~4.7 avg instead of 8): ~40% fewer HBM
    bytes, SWDGE descriptor-generation calls, and vector-engine
    elements.  The slot schedule is derived from the actual input and
    compiled in (nc cache keyed by schedule); outputs are inverse-
    permuted on the host.
  - bilinear corner offsets (int32) and weights (bf16, zeroed on invalid
    slots) are precomputed host-side; valid mask pre-cast to f32.
  - scores via ACT-accumulate (frees DVE); qt/out matmuls in bf16.

Per-tile pipeline (128 queries, K slots):
  layernorm -> xc -> (PE bf16) qt = xc@A + c_qt, qbk = xc@aqbk
  K x indirect DMA -> tg[P, j, (a,b,d)] bf16
  tg *= w4 (4 broadcast multiplies, in place); u = a0+a1; s = b0+b1
  prod = s * qt ; scores[:, j] = ACT-accum(SCALE*prod + qbk/D)
  masked softmax over K slots; g = sum_j attn_j * s_j (pairwise tree)
  out = x + g@Wvo + sum_attn*(bv@Wo) + bo
"""
import math
import numpy as np
from contextlib import ExitStack

import concourse.bass as bass
import concourse.tile as tile
import concourse.mybir as mybir
from concourse.bass import AP
from concourse.masks import make_identity

F32 = mybir.dt.float32
BF16 = mybir.dt.bfloat16
I32 = mybir.dt.int32
ALU = mybir.AluOpType
ACTF = mybir.ActivationFunctionType

# problem constants (hardcoded per harness contract)
B, C, N, D, H, W = 2, 8, 16384, 256, 64, 64
NCORES = 8
ROWS_PER_CORE = (B * N) // NCORES      # 4096
P = 128                                # tile = 128 queries
NTILES = ROWS_PER_CORE // P
EPS = 1e-5
SCALE = 1.0 / math.sqrt(float(D))
BLK = 4 * D                            # 2x2 block = 1024 elements


def build_nc(schedule, rows_per_core=ROWS_PER_CORE):
    """schedule: per-tile slot counts (len == ntiles), each in 1..C."""
    ntiles = rows_per_core // P
    assert len(schedule) == ntiles
    nc = bass.Bass()

    xv_d = nc.dram_tensor("xv", (rows_per_core, D + C + 2), F32,
                          kind="ExternalInput")     # queries | valid | mu, rs
    sa_d = nc.dram_tensor("sa", (rows_per_core,), BF16,
                          kind="ExternalInput")     # sum_attn row (0/1)
    blk_d = nc.dram_tensor("blk", (C * H * W, BLK), BF16,
                           kind="ExternalInput")         # 2x2 pixel blocks
    offs_d = nc.dram_tensor("offs", (rows_per_core, C), I32,
                            kind="ExternalInput")        # block row index
    w4_d = nc.dram_tensor("w4", (rows_per_core, 4 * C), F32,
                          kind="ExternalInput")          # (ab, slot) weights
    wq_d = nc.dram_tensor("Wq", (D, D), F32, kind="ExternalInput")
    bq_d = nc.dram_tensor("bq", (D,), F32, kind="ExternalInput")
    wkv_d = nc.dram_tensor("Wkv", (D, 2 * D), F32, kind="ExternalInput")
    bkv_d = nc.dram_tensor("bkv", (2 * D,), F32, kind="ExternalInput")
    wo_d = nc.dram_tensor("Wo", (D, D), F32, kind="ExternalInput")
    bo_d = nc.dram_tensor("bo", (D,), F32, kind="ExternalInput")
    gamma_d = nc.dram_tensor("gamma", (D,), F32, kind="ExternalInput")
    beta_d = nc.dram_tensor("beta", (D,), F32, kind="ExternalInput")
    out_d = nc.dram_tensor("out", (rows_per_core, D), F32,
                           kind="ExternalOutput")

    with tile.TileContext(nc) as tc, ExitStack() as ctx:
        const = ctx.enter_context(tc.tile_pool(name="const", bufs=1))

        # ---------- constants ----------
        ident = const.tile([P, P], F32)
        make_identity(nc, ident[:])
        ones_row = const.tile([1, D], F32)
        nc.vector.memset(ones_row[:], 1.0)
        epsb = const.tile([P, 1], F32)
        nc.vector.memset(epsb[:], EPS)

        # ---------- persistent weight products ----------
        A_s = const.tile([P, 2, D], BF16)      # diag(gamma) Wq Wk^T (bf16)
        Wvo_s = const.tile([P, 2, D], BF16)    # Wv @ Wo (bf16)
        cqt_row = const.tile([1, D], F32)
        aqbkT = const.tile([P, 2], F32)        # (gamma*Wq@bk) as column slabs
        bvoWo_row = const.tile([1, D], F32)
        bo_row = const.tile([1, D], F32)
        cqbk16_b = const.tile([P, 1], F32)     # (beta@Wq@bk + bq.bk)/sqrt(D) bcast

        with tc.tile_pool(name="setup", bufs=1) as sp, \
             tc.tile_pool(name="setup_ps", bufs=1, space="PSUM") as spp:
            wq_s = sp.tile([P, 2, D], F32)
            nc.sync.dma_start(wq_s[:], wq_d.rearrange("(j p) d -> p j d", p=P))
            wkv_s = sp.tile([P, 2, 2 * D], F32)
            nc.sync.dma_start(wkv_s[:], wkv_d.rearrange("(j p) d -> p j d", p=P))
            wo_s = sp.tile([P, 2, D], F32)
            nc.sync.dma_start(wo_s[:], wo_d.rearrange("(j p) d -> p j d", p=P))
            nc.sync.dma_start(bo_row[:], bo_d[None, :])
            gam_row = sp.tile([1, D], F32)
            nc.sync.dma_start(gam_row[:], gamma_d[None, :])
            bet_row = sp.tile([1, D], F32)
            nc.sync.dma_start(bet_row[:], beta_d[None, :])
            bq_row = sp.tile([1, D], F32)
            nc.sync.dma_start(bq_row[:], bq_d[None, :])
            bkv_row = sp.tile([1, 2 * D], F32)
            nc.sync.dma_start(bkv_row[:], bkv_d[None, :])

            def transpose_256(dst, src):
                for jj in range(2):
                    pst = spp.tile([P, P], F32, tag="tp")
                    for i in range(2):
                        nc.tensor.transpose(
                            pst[:], src[:, i, jj * P:(jj + 1) * P], ident[:])
                        nc.scalar.copy(dst[:, jj, i * P:(i + 1) * P], pst[:])
                        if i == 0:
                            pst = spp.tile([P, P], F32, tag="tp")

            wqT = sp.tile([P, 2, D], F32)
            transpose_256(wqT, wq_s)
            wkT = sp.tile([P, 2, D], F32)
            transpose_256(wkT, wkv_s[:, :, 0:D])
            wvT = sp.tile([P, 2, D], F32)
            transpose_256(wvT, wkv_s[:, :, D:2 * D])

            def col_of(row_ap, tag):
                ps = spp.tile([P, 1], F32, tag="col")
                nc.tensor.transpose(ps[:], row_ap, ident[0:1, 0:1])
                sb = sp.tile([P, 1], F32, tag=tag)
                nc.scalar.copy(sb[:], ps[:])
                return sb

            gcol = [col_of(gam_row[0:1, k * P:(k + 1) * P], f"gcol{k}")
                    for k in range(2)]
            betcol = [col_of(bet_row[0:1, k * P:(k + 1) * P], f"betcol{k}")
                      for k in range(2)]
            bkcol = [col_of(bkv_row[0:1, k * P:(k + 1) * P], f"bkcol{k}")
                     for k in range(2)]
            bvcol = [col_of(bkv_row[0:1, D + k * P:D + (k + 1) * P],
                            f"bvcol{k}") for k in range(2)]

            # A = Wq @ Wk^T (M-blocks i), then scale rows by gamma into A_s
            for i in range(2):
                psA = spp.tile([P, D], F32, tag="mm")
                for k in range(2):
                    nc.tensor.matmul(psA[:], wqT[:, k, i * P:(i + 1) * P],
                                     wkT[:, k, :], start=(k == 0),
                                     stop=(k == 1))
                nc.vector.tensor_scalar(A_s[:, i, :], psA[:], gcol[i][:],
                                        None, ALU.mult)

            # W_vo = Wv @ Wo
            for i in range(2):
                psV = spp.tile([P, D], F32, tag="mm")
                for k in range(2):
                    nc.tensor.matmul(psV[:], wvT[:, k, i * P:(i + 1) * P],
                                     wo_s[:, k, :], start=(k == 0),
                                     stop=(k == 1))
                nc.scalar.copy(Wvo_s[:, i, :], psV[:])

            # u = beta@Wq + bq  (row)
            psu = spp.tile([1, D], F32, tag="row")
            for k in range(2):
                nc.tensor.matmul(psu[:], betcol[k][:], wq_s[:, k, :],
                                 start=(k == 0), stop=False)
            nc.tensor.matmul(psu[:], ones_row[0:1, 0:1], bq_row[:],
                             start=False, stop=True)
            u_row = sp.tile([1, D], F32)
            nc.scalar.copy(u_row[:], psu[:])
            ucol = [col_of(u_row[0:1, k * P:(k + 1) * P], f"ucol{k}")
                    for k in range(2)]

            # c_qt = u @ Wk^T
            psc = spp.tile([1, D], F32, tag="row")
            for k in range(2):
                nc.tensor.matmul(psc[:], ucol[k][:], wkT[:, k, :],
                                 start=(k == 0), stop=(k == 1))
            nc.scalar.copy(cqt_row[:], psc[:])

            # wqbk = bk^T @ Wq^T (row);  a_qbk = gamma * wqbk
            psw = spp.tile([1, D], F32, tag="row")
            for k in range(2):
                nc.tensor.matmul(psw[:], bkcol[k][:], wqT[:, k, :],
                                 start=(k == 0), stop=(k == 1))
            wqbk_row = sp.tile([1, D], F32)
            nc.scalar.copy(wqbk_row[:], psw[:])
            aqbk_row = sp.tile([1, D], F32)
            nc.vector.tensor_tensor(aqbk_row[:], wqbk_row[:], gam_row[:],
                                    ALU.mult)
            for k in range(2):
                psq = spp.tile([P, 1], F32, tag="col")
                nc.tensor.transpose(psq[:], aqbk_row[0:1, k * P:(k + 1) * P],
                                    ident[0:1, 0:1])
                nc.scalar.copy(aqbkT[:, k:k + 1], psq[:])

            # c_qbk = beta.wqbk + bq.bk  -> broadcast (x 1/sqrt(D)) to [P,1]
            scr_row = sp.tile([1, D], F32)
            nc.vector.tensor_tensor(scr_row[:], bet_row[:], wqbk_row[:],
                                    ALU.mult)
            cq1 = sp.tile([1, 1], F32)
            nc.vector.tensor_reduce(cq1[:], scr_row[:], mybir.AxisListType.X,
                                    ALU.add)
            scr2_row = sp.tile([1, D], F32)
            nc.vector.tensor_tensor(scr2_row[:], bq_row[:], bkv_row[0:1, 0:D],
                                    ALU.mult)
            cq2 = sp.tile([1, 1], F32)
            nc.vector.tensor_reduce(cq2[:], scr2_row[:], mybir.AxisListType.X,
                                    ALU.add)
            cq16 = sp.tile([1, 1], F32)
            nc.vector.tensor_tensor(cq16[:], cq1[:], cq2[:], ALU.add)
            nc.vector.tensor_scalar(cq16[:], cq16[:], SCALE, None, ALU.mult)
            psb = spp.tile([P, 1], F32, tag="col")
            nc.tensor.matmul(psb[:], ones_row[0:1, 0:P], cq16[:],
                             start=True, stop=True)
            nc.scalar.copy(cqbk16_b[:], psb[:])

            # b_voWo = bv @ Wo (row)
            psv = spp.tile([1, D], F32, tag="row")
            for k in range(2):
                nc.tensor.matmul(psv[:], bvcol[k][:], wo_s[:, k, :],
                                 start=(k == 0), stop=(k == 1))
            nc.scalar.copy(bvoWo_row[:], psv[:])

        # bf16 copies of row constants for the bf16 matmul chains
        cqt_row16 = const.tile([1, D], BF16)
        nc.vector.tensor_copy(cqt_row16[:], cqt_row[:])
        bvoWo_row16 = const.tile([1, D], BF16)
        nc.vector.tensor_copy(bvoWo_row16[:], bvoWo_row[:])
        bo_row16 = const.tile([1, D], BF16)
        nc.vector.tensor_copy(bo_row16[:], bo_row[:])
        ones_row16 = const.tile([1, D], BF16)
        nc.vector.tensor_copy(ones_row16[:], ones_row[:])
        ident16 = const.tile([P, P], BF16)
        nc.vector.tensor_copy(ident16[:], ident[:])
        aqbkT16 = const.tile([P, 2], BF16)
        nc.vector.tensor_copy(aqbkT16[:], aqbkT[:])

        # ---------- main loop pools ----------
        big = ctx.enter_context(tc.tile_pool(name="big", bufs=2))
        mid = ctx.enter_context(tc.tile_pool(name="mid", bufs=2))
        sml = ctx.enter_context(tc.tile_pool(name="sml", bufs=4))
        ps_early = ctx.enter_context(
            tc.tile_pool(name="ps_early", bufs=2, space="PSUM"))
        ps_qt = ctx.enter_context(
            tc.tile_pool(name="ps_qt", bufs=2, space="PSUM"))
        ps_late = ctx.enter_context(
            tc.tile_pool(name="ps_late", bufs=2, space="PSUM"))
        ps_out = ctx.enter_context(
            tc.tile_pool(name="ps_out", bufs=2, space="PSUM"))

        for it in range(ntiles):
            n0 = it * P
            K = int(schedule[it])
            # ---- loads ----
            xvt = sml.tile([P, D + C + 2], F32, tag="xvt")
            nc.sync.dma_start(xvt[:], xv_d[n0:n0 + P, :])
            xt = xvt[:, 0:D]
            valid_f = xvt[:, D:D + K]
            mu = xvt[:, D + C:D + C + 1]
            rs = xvt[:, D + C + 1:D + C + 2]
            offt = sml.tile([P, C], I32, tag="offt")
            nc.sync.dma_start(offt[:], offs_d[n0:n0 + P, :])
            w4t = sml.tile([P, 4 * C], F32, tag="w4t")
            nc.sync.dma_start(w4t[:], w4_d[n0:n0 + P, :])

            # ---- the gather: one 2KB block per (query, slot) ----
            tg = big.tile([P, C, BLK], BF16, tag="tg", name="tg")
            for j in range(K):
                nc.gpsimd.indirect_dma_start(
                    out=tg[:, j, :],
                    out_offset=None, in_=blk_d[:],
                    in_offset=bass.IndirectOffsetOnAxis(
                        ap=offt[:, j:j + 1], axis=0))

            # ---- layernorm (host-computed mu/rs) + qt (bf16 PE) ----
            xc = sml.tile([P, D], F32, tag="xc")
            nc.vector.tensor_scalar(xc[:], xt, mu, rs,
                                    ALU.subtract, ALU.mult)
            pse = ps_early.tile([P, 2 * P + 1], F32, tag="pse")
            xcT_ps = pse[:, 0:2 * P]
            qbk_ps = pse[:, 2 * P:2 * P + 1]
            for j in range(2):
                nc.tensor.transpose(xcT_ps[:, j * P:(j + 1) * P],
                                    xc[:, j * P:(j + 1) * P], ident[:])
            xcT = sml.tile([P, 2 * P], BF16, tag="xcT")
            nc.scalar.copy(xcT[:], xcT_ps)

            qt_ps = ps_qt.tile([P, D], F32, tag="qt")
            for j in range(2):
                nc.tensor.matmul(qt_ps[:], xcT[:, j * P:(j + 1) * P],
                                 A_s[:, j, :], start=(j == 0), stop=False)
            nc.tensor.matmul(qt_ps[:], ones_row16[0:1, 0:P], cqt_row16[:],
                             start=False, stop=True)
            for j in range(2):
                nc.tensor.matmul(qbk_ps, xcT[:, j * P:(j + 1) * P],
                                 aqbkT16[:, j:j + 1], start=(j == 0),
                                 stop=(j == 1))
            qt_sb = sml.tile([P, D], BF16, tag="qtsb")
            nc.scalar.copy(qt_sb[:], qt_ps[:])
            # qbk score bias: qbk*SCALE + cqbk16
            qbk_col = sml.tile([P, 1], F32, tag="qbkcol")
            nc.scalar.copy(qbk_col[:], qbk_ps)
            nc.vector.tensor_scalar(qbk_col[:], qbk_col[:], SCALE,
                                    cqbk16_b[:], ALU.mult, ALU.add)

            # ---- bilinear: weight the 4 corners (in place), then reduce ----
            for ab in range(4):
                wb = w4t[:, ab * C:ab * C + K].rearrange(
                    "p (c a) -> p c a", a=1).to_broadcast([P, K, D])
                seg = tg[:, 0:K, ab * D:(ab + 1) * D]
                nc.vector.tensor_tensor(seg, seg, wb, ALU.mult)
            nc.vector.tensor_tensor(tg[:, 0:K, 0:2 * D], tg[:, 0:K, 0:2 * D],
                                    tg[:, 0:K, 2 * D:4 * D], ALU.add)
            s = mid.tile([P, C, D], BF16, tag="s")
            nc.vector.tensor_tensor(s[:, 0:K, :], tg[:, 0:K, 0:D],
                                    tg[:, 0:K, D:2 * D], ALU.add)

            # ---- scores: ACT Copy-accumulate of SCALE*prod, then +qbk ----
            prod = mid.tile([P, C, D], BF16, tag="prod")
            qt_b = qt_sb.rearrange("p (a d) -> p a d", a=1).to_broadcast(
                [P, K, D])
            nc.vector.tensor_tensor(prod[:, 0:K, :], s[:, 0:K, :], qt_b,
                                    ALU.mult)
            scores = sml.tile([P, C], F32, tag="scores")
            scr = sml.tile([P, D], F32, tag="actscr")
            for j in range(K):
                nc.scalar.activation(scr[:], prod[:, j, :], ACTF.Copy,
                                     bias=0.0, scale=SCALE,
                                     accum_out=scores[:, j:j + 1])

            # ---- masked softmax over K slots (unnormalized value path) ----
            # masked = (scores + qbk) * valid + (valid*1e30 - 1e30)
            vneg = sml.tile([P, C], F32, tag="vneg")
            nc.scalar.activation(vneg[:, 0:K], valid_f, ACTF.Copy,
                                 bias=-1e30, scale=1e30)
            masked = sml.tile([P, C], F32, tag="masked")
            nc.vector.scalar_tensor_tensor(masked[:, 0:K], scores[:, 0:K],
                                           qbk_col[:], valid_f,
                                           ALU.add, ALU.mult)
            nc.vector.tensor_tensor(masked[:, 0:K], masked[:, 0:K],
                                    vneg[:, 0:K], ALU.add)
            negm = sml.tile([P, 1], F32, tag="negm")
            nc.vector.tensor_reduce(negm[:], masked[:, 0:K],
                                    mybir.AxisListType.X, ALU.max, negate=True)
            expd = sml.tile([P, C], F32, tag="expd")
            nc.scalar.activation(expd[:, 0:K], masked[:, 0:K], ACTF.Exp,
                                 bias=negm[:], scale=1.0)
            ssum = sml.tile([P, 1], F32, tag="ssum")
            nc.vector.tensor_reduce(ssum[:], expd[:, 0:K],
                                    mybir.AxisListType.X, ALU.add)
            nc.vector.tensor_scalar(ssum[:], ssum[:], 1e-30, None, ALU.add)
            nc.vector.reciprocal(ssum[:], ssum[:])

            # ---- g_raw = sum_j expd_j * s_j ; 1/ssum folds into gT copy ----
            sw = mid.tile([P, C, D], BF16, tag="sw")
            expd_b = expd.rearrange("p (c a) -> p c a", a=1)[:, 0:K, :] \
                .to_broadcast([P, K, D])
            nc.vector.tensor_tensor(sw[:, 0:K, :], s[:, 0:K, :], expd_b,
                                    ALU.mult)
            k = K
            while k > 2:
                h = (k + 1) // 2
                nc.vector.tensor_tensor(sw[:, 0:k - h, :], sw[:, 0:k - h, :],
                                        sw[:, h:k, :], ALU.add)
                k = h
            g = sml.tile([P, D], F32, tag="g")
            if k == 2:
                nc.vector.tensor_tensor(g[:], sw[:, 0, :], sw[:, 1, :],
                                        ALU.add)
            else:
                nc.vector.tensor_copy(g[:], sw[:, 0, :])
            nc.vector.tensor_scalar(g[:], g[:], ssum[:], None, ALU.mult)

            # ---- final: out = x + g@Wvo + sum_attn*bvoWo + bo (bf16 PE) ----
            psl = ps_late.tile([P, 2 * P], F32, tag="psl")
            gT_ps = psl[:, 0:2 * P]
            for j in range(2):
                nc.tensor.transpose(gT_ps[:, j * P:(j + 1) * P],
                                    g[:, j * P:(j + 1) * P], ident[:])
            gT = sml.tile([P, 2 * P], BF16, tag="gT")
            nc.scalar.copy(gT[:], gT_ps)
            saT = sml.tile([1, P], BF16, tag="saT")
            nc.sync.dma_start(saT[:], sa_d[None, n0:n0 + P])

            out_ps = ps_out.tile([P, D], F32, tag="out")
            for j in range(2):
                nc.tensor.matmul(out_ps[:], gT[:, j * P:(j + 1) * P],
                                 Wvo_s[:, j, :], start=(j == 0), stop=False)
            nc.tensor.matmul(out_ps[:], saT[:], bvoWo_row16[:], start=False,
                             stop=False)
            nc.tensor.matmul(out_ps[:], ones_row16[0:1, 0:P], bo_row16[:],
                             start=False, stop=True)
            out_sb = sml.tile([P, D], F32, tag="outsb")
            nc.vector.tensor_tensor(out_sb[:], out_ps[:], xt, ALU.add)
            nc.sync.dma_start(out_d[n0:n0 + P, :], out_sb[:])

    return nc


# ---------------------------------------------------------------------------
# Post-scheduling legalization: the walrus build here encodes at most ONE
# sync-wait command per TPB instruction (matmul LDWEIGHTS / CTRL structs
# reject more). Hoist excess waits onto same-engine EventSemaphore helpers
# inserted immediately before the offending instruction (sequencer order
# preserves blocking semantics exactly).
_LGL_UID = [0]


def legalize_waits(nc, cap=1):
    n_helpers = 0
    for fn in nc.m.functions:
        for bb in fn.blocks:
            out = []
            for ins in bb.instructions:
                si = ins.sync_info
                waits = list(si.on_wait) if si is not None else []
                if len(waits) > cap:
                    excess, keep = waits[:-cap], waits[-cap:]
                    for w in excess:
                        _LGL_UID[0] += 1
                        helper = mybir.InstEventSemaphore(
                            name=f"I-lgl-{_LGL_UID[0]}", ins=[], outs=[])
                        helper.engine = ins.engine
                        helper.sync_info = mybir.SyncInfo(
                            on_wait=[w], on_update=[])
                        out.append(helper)
                        n_helpers += 1
                    ins.sync_info = mybir.SyncInfo(
                        on_wait=keep,
                        on_update=list(si.on_update) if si is not None else [])
                out.append(ins)
            bb.instructions = out
    return n_helpers


_NC_CACHE = {}


def _get_nc(schedule, rows_per_core=ROWS_PER_CORE):
    key = (tuple(schedule), rows_per_core)
    if key not in _NC_CACHE:
        nc = build_nc(schedule, rows_per_core)
        legalize_waits(nc)
        _NC_CACHE[key] = nc
    return _NC_CACHE[key]


def prepare(inputs, rows_per_core=ROWS_PER_CORE, ncores=NCORES):
    """Host marshalling: sort + compact + schedule.

    Returns (in_maps, schedules, inv_orders).  All cores share ONE compiled
    kernel, so the schedule is the per-tile MAX slot count across cores.
    """
    import ml_dtypes

    q = np.ascontiguousarray(np.asarray(inputs["queries"], np.float32))
    feat = np.ascontiguousarray(
        np.asarray(inputs["image_features"], np.float32))
    pc = np.ascontiguousarray(np.asarray(inputs["pixel_coords"], np.float32))
    vm = np.ascontiguousarray(np.asarray(inputs["valid_mask"], np.int32))
    wshared = {
        "Wq": np.ascontiguousarray(np.asarray(inputs["Wq"], np.float32)),
        "bq": np.ascontiguousarray(np.asarray(inputs["bq"], np.float32)),
        "Wkv": np.ascontiguousarray(np.asarray(inputs["Wkv"], np.float32)),
        "bkv": np.ascontiguousarray(np.asarray(inputs["bkv"], np.float32)),
        "Wo": np.ascontiguousarray(np.asarray(inputs["Wo"], np.float32)),
        "bo": np.ascontiguousarray(np.asarray(inputs["bo"], np.float32)),
        "gamma": np.ascontiguousarray(np.asarray(inputs["gamma"], np.float32)),
        "beta": np.ascontiguousarray(np.asarray(inputs["beta"], np.float32)),
    }

    # --- 2x2 overlapping pixel blocks in bf16: (B, C*H*W, 4*D) ---
    fp = np.pad(feat, ((0, 0), (0, 0), (0, 1), (0, 1), (0, 0)), mode="edge")
    win = np.lib.stride_tricks.sliding_window_view(fp, (2, 2), axis=(2, 3))
    blk = win.transpose(0, 1, 2, 3, 5, 6, 4).astype(
        ml_dtypes.bfloat16).reshape(B, C * H * W, BLK)

    # --- bilinear offsets + weights (query-major, camera axis last) ---
    p = (pc + np.float32(1.0)) * np.float32(31.5)        # (B, C, N, 2) f32
    p0 = np.minimum(np.floor(p), np.float32(W - 2))
    fr = p - p0
    x0 = p0[..., 0].astype(np.int32)
    y0 = p0[..., 1].astype(np.int32)
    fx = fr[..., 0]
    fy = fr[..., 1]
    cam = (np.arange(C, dtype=np.int32) * (H * W))[None, :, None]
    offs = (cam + y0 * W + x0).astype(np.int32)          # (B, C, N)
    w_ab = np.stack([(1 - fy) * (1 - fx), (1 - fy) * fx,
                     fy * (1 - fx), fy * fx], axis=1)    # (B, 4, C, N)

    # layernorm stats on host (f32, matches device tolerance)
    mu_all = q.mean(-1, keepdims=True).astype(np.float32)        # (B, N, 1)
    var_all = q.var(-1, keepdims=True).astype(np.float32)
    rs_all = (1.0 / np.sqrt(var_all + np.float32(EPS))).astype(np.float32)

    ntiles = rows_per_core // P
    per_b = N // (ncores // B)
    in_maps = []
    inv_orders = []
    Kmat = np.zeros((ncores, ntiles), np.int32)
    core_data = []
    for core in range(ncores):
        b = core // (ncores // B)
        n0 = (core % (ncores // B)) * per_b
        sl = slice(n0, n0 + rows_per_core)
        v = vm[b, :, sl]                          # (C, rows)
        kcnt = v.sum(0)                           # (rows,)
        order = np.argsort(kcnt, kind="stable")
        inv_orders.append(np.argsort(order))
        vs = v[:, order].T                        # (rows, C) sorted queries
        # valid cameras first (stable -> ascending cam id among valid)
        slot_cam = np.argsort(-vs, axis=1, kind="stable")  # (rows, C)
        take = lambda a: np.take_along_axis(a, slot_cam, axis=1)
        offs_s = take(offs[b, :, sl].T[order])             # (rows, C)
        valid_s = take(vs).astype(np.float32)              # (rows, C)
        w_s = np.stack([take(w_ab[b, ab, :, sl].T[order])
                        for ab in range(4)], axis=1)       # (rows, 4, C)
        w_s *= valid_s[:, None, :]
        offs_s = offs_s * (valid_s != 0)                   # padding -> block 0
        kq = vs.sum(1)                                     # sorted counts
        Kmat[core] = np.maximum(
            kq.reshape(ntiles, P).max(axis=1), 1)
        xq = q[b, sl][order]
        murs = np.concatenate([mu_all[b, sl], rs_all[b, sl]], axis=-1)[order]
        sa = (kq > 0)
        core_data.append((xq, valid_s, murs, sa, offs_s.astype(np.int32),
                          w_s.reshape(rows_per_core, 4 * C), b))
    schedule = Kmat.max(axis=0)

    import ml_dtypes
    for xq, valid_s, murs, sa, offs_s, w_s, b in core_data:
        m = {
            "xv": np.ascontiguousarray(
                np.concatenate([xq, valid_s, murs], axis=-1)),
            "sa": np.ascontiguousarray(sa.astype(ml_dtypes.bfloat16)),
            "blk": blk[b],
            "offs": np.ascontiguousarray(offs_s),
            "w4": np.ascontiguousarray(w_s.astype(np.float32)),
        }
        m.update(wshared)
        in_maps.append(m)
    return in_maps, schedule, inv_orders


def kernel(**inputs) -> np.ndarray:
    from concourse.bass_utils import run_bass_kernel_spmd
    in_maps, schedule, inv_orders = prepare(inputs)
    nc = _get_nc(schedule)
    res = run_bass_kernel_spmd(nc, in_maps, core_ids=list(range(NCORES)))
    outs = [np.asarray(r["out"])[inv_orders[c]]
            for c, r in enumerate(res.results)]
    full = np.concatenate(outs, axis=0).reshape(B, N, D)
    return full.astype(np.float32)
